# revision 13
# baseline (speedup 1.0000x reference)
"""Trainium2 Bass kernel for nn_AttentionBasedModulator.

Computes out[b, n, c, h, w] = query_features[b, c, h, w]
                              * support_fibers[c, n] * diag_weight[c]

Sharding: data-parallel over batch B=8, one batch element per NeuronCore.

The kernel is DMA-write bound: the output is 32x larger than the input
(pure broadcast expansion), so per core the fp32 output would be
48 MiB vs ~1.5 MiB of input. The device computes and stores the output
in bf16 (24 MiB per core; worst-case rel err ~0.8%, measured 0.0077 vs
the 2e-2 gate) and the host upcasts to fp32 while gathering. At bf16
the steady state sits exactly on the HBM-per-NeuronCore write limit
(~358 GB/s = 716 GB/s per HBM stack shared by 2 cores): 24 MiB /
70 us = 359 GB/s. Three probes pin this as the physical roofline: a
pure-DMA store loop (dma_pure) times identically, fp32 output times
exactly 2x, and every layout/queue variant (gpd 2/4/8, dual-ring,
dup-layouts) lands in the same 69-72 us noise band. fp8 would halve
the bytes but cannot pass the gate (e4m3 rounding alone is 6.25%).

Primary builder build_nat (natural layout):
  - partition p holds channels 3p..3p+2 (CS = C/128 = 3); no q
    duplication, no SBUF replication step.
  - q arrives from the host already cast to bf16 (make_in_maps;
    identical rounding to the on-device cast it replaces), halving the
    q read and removing the cast from the prologue's critical path.
  - s tile [128, 3, 32] fp32: st[p, k, n] = sf[3p+k, n] * dw[3p+k],
    loaded via strided view, scaled on-chip. Kept fp32 (the
    tensor_scalar scalar operand may be fp32 in any DVE perf mode).
  - one output DMA covers gpd=4 prototypes through a 4-dim DRAM view
    out[gt, gp, g4, cs, f]: 3 MiB per DMA, 2 KiB contiguous per
    (partition, g4, cs) run — comfortably above the ~0.7 KiB where
    descriptor overhead would eat the 435 (fabric) / 358 (HBM) slack.
  - per tile: 12 DVE tensor_scalar multiplies (bf16 4x perf mode) into
    a bf16 tile, then the store; 96 multiplies ~18 us, hidden under
    70 us of DMA.
  - fine_edges: first/last tile stored in per-(g4, cs) 0.25 MiB steps,
    so the first store issues ~2 us into the program (after one 0.25 MiB
    q load + one multiply) and the final drain is short. The full-body
    probe (q load + all stores per iteration, inner=1 so the For_i
    back-edge exposes fill/drain) times the real single-shot NEFF body
    at ~79 us vs ~103 us for the legacy dup-layout build, whose
    whole-tile SBUF replication barrier serialized ~18 us of prologue.

The legacy dup-layout build() is kept as the fallback path.
"""

import numpy as np

C, NP = 384, 32          # channels, prototypes
B, H, W = 8, 32, 32
HW = H * W
P = 128                  # SBUF partitions
N_CORES = 8
DUP = 4                  # q duplication factor / prototypes per output DMA
BUFS = 4                 # output tile slots
ACT_SPLIT = 0            # of each group's CSD multiplies, how many go
                         # to the ACT (scalar) engine instead of the DVE


def build(repeat: int = 1, timing: bool = False, outer: int = 0,
          dup: int = DUP, onchip_dup: bool = True,
          bufs: int = BUFS, act_split: int = ACT_SPLIT, dma_lite: bool = False,
          scratch_regions: int = 5,
          dma_pure: bool = False, dual_ring: bool = False,
          split_half: bool = False, full_body: bool = False,
          fine_edges: bool = True, out_f32: bool = False):
    """Build and compile the Bass program for one core.

    timing=True: each repeat writes a distinct Internal DRAM region (so
    stores cannot be dead-store-eliminated); a final DRAM->DRAM readback
    of a few bytes per region forms the only ExternalOutput, so dispatch
    timing is not dominated by fetching the full output to the host.
    outer=N (with timing): wraps the python-unrolled `repeat` body in a
    hardware For_i loop of N iterations, giving N*repeat total repeats at
    fixed compile cost - long device programs make wall-clock timing
    robust to dispatch-overhead noise. The loop back-edge costs one
    drain+barrier per `repeat` repeats (amortized, slightly conservative).
    dup: q duplication factor (1, 2, or 4) - prototypes per output DMA;
    per-partition contiguous DRAM runs are dup*3 KiB (bf16).
    onchip_dup=True: load q from HBM once (1.5 MiB) into partition block
    0 and replicate across the dup blocks by log2-doubling SBUF->SBUF
    DMA copies, instead of dup HBM loads - cuts the real (repeat=1)
    kernel's HBM read traffic by (dup-1)*1.5 MiB; no effect on the
    steady-state repeat loop.
    dma_lite=True: only one multiply per output tile (rest of the tile is
    stale slot data) - isolates DMA-write throughput from DVE work.
    out_f32=True: fp32 output path (for A/B against the bf16 one).
    """
    import concourse.bacc as bacc
    import concourse.mybir as mybir
    from concourse.tile import TileContext

    nc = bacc.Bacc(None, target_bir_lowering=False)
    f32 = mybir.dt.float32
    bf16 = mybir.dt.bfloat16
    odt = f32 if out_f32 else bf16
    act_copy = mybir.ActivationFunctionType.Copy

    GP = P // dup            # partition groups (64 for dup=2)
    CSD = C // GP            # channels per partition (6 for dup=2)
    NPG = NP // dup          # prototype groups = output DMAs per repeat

    q = nc.dram_tensor("q", [C, HW], f32, kind="ExternalInput")
    sf = nc.dram_tensor("sf", [C, NP], f32, kind="ExternalInput")
    dw = nc.dram_tensor("dw", [C, 1], f32, kind="ExternalInput")
    if timing:
        nreg = min(repeat, scratch_regions)
        scratch = nc.dram_tensor("scratch", [nreg, NP, C, HW], odt,
                                 kind="Internal")
        tiny = nc.dram_tensor("out", [nreg, 4], odt, kind="ExternalOutput")
        out_views = [scratch[r % nreg] for r in range(repeat)]
    else:
        out = nc.dram_tensor("out", [NP, C, HW], odt, kind="ExternalOutput")
        tiny = None
        out_views = [out] * repeat

    # Grouped views: partition p = d*GP + gp <-> channels CSD*gp..CSD*gp+CSD-1
    # (the d halves hold IDENTICAL q channels but serve prototypes n = g*dup+d).
    q_r = q.rearrange("(gp cs) f -> gp cs f", cs=CSD)        # [GP, CSD, 1024]
    sf_r = sf.rearrange("(gp cs) (g d) -> d gp cs g",
                        cs=CSD, d=dup)                       # [dup,GP,CSD,NPG]
    dw_r = dw.rearrange("(gp cs) o -> gp cs o", cs=CSD)      # [GP, CSD, 1]

    with TileContext(nc) as tc:
        with tc.tile_pool(name="consts", bufs=1) as cpool, \
             tc.tile_pool(name="qpool", bufs=1) as qpool, \
             tc.tile_pool(name="work", bufs=bufs) as wpool:
            # Tiny sf/dw loads first: the s precompute overlaps the q load.
            st = cpool.tile([P, CSD, NPG], f32, name="st")
            for d in range(dup):
                nc.sync.dma_start(out=st[d * GP:(d + 1) * GP], in_=sf_r[d])
            dt_ = cpool.tile([P, CSD], f32, name="dt")
            for d in range(dup):
                nc.sync.dma_start(out=dt_[d * GP:(d + 1) * GP], in_=dw_r)

            def load_q():
                qt = qpool.tile([P, CSD, HW], f32, name="qt", tag="qt")
                qb = qt if out_f32 else qpool.tile([P, CSD, HW], bf16,
                                                   name="qb", tag="qb")
                if onchip_dup:
                    # One 1.5 MiB HBM load into partition block 0 (per-cs
                    # so casts overlap the loads), then replicate to the
                    # other dup-1 blocks by doubling SBUF->SBUF copies.
                    for cs in range(CSD):
                        nc.sync.dma_start(out=qt[0:GP, cs, :],
                                          in_=q_r[:, cs, :])
                        if not out_f32:
                            nc.vector.tensor_scalar_mul(qb[0:GP, cs, :],
                                                        qt[0:GP, cs, :], 1.0)
                    blk = GP
                    while blk < P:
                        nc.sync.dma_start(out=qb[blk:2 * blk], in_=qb[0:blk])
                        blk *= 2
                else:
                    # Per-(d, cs) loads let the first multiplies start
                    # sooner.
                    for cs in range(CSD):
                        for d in range(dup):
                            nc.sync.dma_start(
                                out=qt[d * GP:(d + 1) * GP, cs, :],
                                in_=q_r[:, cs, :])
                        if not out_f32:
                            nc.vector.tensor_scalar_mul(qb[:, cs, :],
                                                        qt[:, cs, :], 1.0)
                return qb

            for cs in range(CSD):
                nc.vector.tensor_scalar_mul(st[:, cs, :], st[:, cs, :],
                                            dt_[:, cs:cs + 1])

            qb = None if full_body else load_q()

            src = None
            if dma_pure:
                # One static source tile, filled once: the repeat loop is
                # pure independent DMA stores (measures the DMA ceiling).
                src = cpool.tile([P, CSD, HW], odt, name="src")
                for cs in range(CSD):
                    nc.vector.tensor_scalar_mul(
                        src[:, cs, :], qb[:, cs, :], st[:, cs, 0:1])

            def emit_repeats():
                for r in range(repeat):
                    emit_one(r, load_q() if full_body else qb)

            def emit_one(r, qb):
                # One DMA covers prototypes n = g*dup..g*dup+dup-1: partition
                # p = d*GP+gp writes the contiguous CSD-channel run of
                # prototype g*dup+d -> a single contiguous dup*768 KiB span.
                out_r = out_views[r].rearrange(
                    "(g d) (gp cs) f -> g (d gp) cs f", d=dup, cs=CSD)
                for g in range(NPG):
                    dma_eng = nc.scalar if (dual_ring and g % 2) else nc.sync
                    if dma_pure:
                        dma_eng.dma_start(out=out_r[g], in_=src[:])
                        continue
                    ot = wpool.tile([P, CSD, HW], odt, name="ot", tag="ot")
                    for cs in range(CSD):
                        if dma_lite and cs > 0:
                            continue
                        if cs < act_split:
                            nc.scalar.activation(
                                ot[:, cs, :], qb[:, cs, :], act_copy,
                                scale=st[:, cs, g:g + 1])
                        else:
                            nc.vector.tensor_scalar_mul(
                                ot[:, cs, :], qb[:, cs, :],
                                st[:, cs, g:g + 1])
                    if fine_edges and g in (0, NPG - 1) and not dma_lite:
                        # Fill/drain the pipeline in per-cs steps at the
                        # kernel edges: the first DMA starts after one
                        # multiply instead of CSD, and the final drain is
                        # 1/CSD as long.
                        for cs in range(CSD):
                            dma_eng.dma_start(out=out_r[g][:, cs, :],
                                              in_=ot[:, cs, :])
                    elif split_half:
                        # Same tile as two concurrent half-DMAs, one per
                        # HWDGE ring (partition halves map to disjoint
                        # SDMA engine sets).
                        nc.sync.dma_start(out=out_r[g][0:P // 2],
                                          in_=ot[0:P // 2])
                        nc.scalar.dma_start(out=out_r[g][P // 2:P],
                                            in_=ot[P // 2:P])
                    else:
                        dma_eng.dma_start(out=out_r[g], in_=ot[:])

            if timing and outer:
                with tc.For_i(0, outer, 1):
                    emit_repeats()
            else:
                emit_repeats()

            if timing:
                nc.sync.dma_start(out=tiny[:], in_=scratch[:, 0, 0, 0:4])

    nc.compile()
    return nc


GPD = 4                  # prototypes per output DMA tile (natural layout)
NAT_BUFS = 4


def build_nat(repeat: int = 1, timing: bool = False, outer: int = 0,
              gpd: int = GPD, bufs: int = NAT_BUFS,
              fine_edges: bool = True, full_body: bool = False,
              scratch_regions: int = 5, dma_pure: bool = False,
              dual_ring: bool = False, out_f32: bool = False,
              q_bf16: bool = True):
    """Natural-layout builder: partition p holds channels 3p..3p+2 (no q
    duplication), one output DMA covers `gpd` prototypes via a 4-dim DRAM
    access pattern out[gt, gp, g4, cs, f] (per-partition contiguous runs
    of 2 KiB at bf16 — well above the ~0.7 KiB where descriptor overhead
    would start to eat into the 435/358 fabric/HBM slack).

    The point vs the dup-layout build(): the prologue is a per-cs
    load -> cast -> (first-tile multiply + store) dataflow chain with no
    whole-tile replication barrier, so the first output DMA issues ~3 us
    into the program instead of ~18 us. Steady state is identical (HBM
    write roofline).

    full_body=True (timing only): the q load + cast runs inside every
    repeat, so with an INNER=1 hardware loop the per-iteration marginal
    time ~= the real single-shot NEFF body time (fill/drain included,
    exposed by the For_i back-edge drain).
    """
    import concourse.bacc as bacc
    import concourse.mybir as mybir
    from concourse.tile import TileContext

    nc = bacc.Bacc(None, target_bir_lowering=False)
    f32 = mybir.dt.float32
    bf16 = mybir.dt.bfloat16
    odt = f32 if out_f32 else bf16

    CS = C // P              # 3 channels per partition
    NT = NP // gpd           # output DMA tiles per repeat

    if out_f32:
        q_bf16 = False
    # q arrives pre-cast to bf16 by make_in_maps (identical rounding to
    # the on-device cast it replaces): halves the q read and drops the
    # cast + f32 staging tile from the prologue's critical path.
    q = nc.dram_tensor("q", [C, HW], bf16 if q_bf16 else f32,
                       kind="ExternalInput")
    sf = nc.dram_tensor("sf", [C, NP], f32, kind="ExternalInput")
    dw = nc.dram_tensor("dw", [C, 1], f32, kind="ExternalInput")
    if timing:
        nreg = min(repeat, scratch_regions)
        scratch = nc.dram_tensor("scratch", [nreg, NP, C, HW], odt,
                                 kind="Internal")
        tiny = nc.dram_tensor("out", [nreg, 4], odt, kind="ExternalOutput")
        out_views = [scratch[r % nreg] for r in range(repeat)]
    else:
        out = nc.dram_tensor("out", [NP, C, HW], odt, kind="ExternalOutput")
        tiny = None
        out_views = [out] * repeat

    q_r = q.rearrange("(gp cs) f -> gp cs f", cs=CS)        # [128, 3, 1024]
    sf_r = sf.rearrange("(gp cs) n -> gp cs n", cs=CS)      # [128, 3, 32]
    dw_r = dw.rearrange("(gp cs) o -> gp cs o", cs=CS)      # [128, 3, 1]

    with TileContext(nc) as tc:
        with tc.tile_pool(name="consts", bufs=1) as cpool, \
             tc.tile_pool(name="qpool", bufs=(2 if full_body else 1)) as qpool, \
             tc.tile_pool(name="work", bufs=bufs) as wpool:
            st = cpool.tile([P, CS, NP], f32, name="st")
            nc.sync.dma_start(out=st[:], in_=sf_r)
            dt_ = cpool.tile([P, CS], f32, name="dt")
            nc.sync.dma_start(out=dt_[:], in_=dw_r)
            for cs in range(CS):
                nc.vector.tensor_scalar_mul(st[:, cs, :], st[:, cs, :],
                                            dt_[:, cs:cs + 1])

            def load_q():
                # Per-cs loads (+casts if q arrives f32): each is an
                # independent dataflow chain, so tile 0's multiply/store
                # for cs starts as soon as THAT cs has landed.
                if q_bf16:
                    qb = qpool.tile([P, CS, HW], bf16, name="qb", tag="qb")
                    for cs in range(CS):
                        nc.sync.dma_start(out=qb[:, cs, :], in_=q_r[:, cs, :])
                    return qb
                qt = qpool.tile([P, CS, HW], f32, name="qt", tag="qt")
                qb = qt if out_f32 else qpool.tile([P, CS, HW], bf16,
                                                   name="qb", tag="qb")
                for cs in range(CS):
                    nc.sync.dma_start(out=qt[:, cs, :], in_=q_r[:, cs, :])
                    if not out_f32:
                        nc.vector.tensor_scalar_mul(qb[:, cs, :],
                                                    qt[:, cs, :], 1.0)
                return qb

            qb = None if full_body else load_q()

            src = None
            if dma_pure:
                src = cpool.tile([P, gpd, CS, HW], odt, name="src")
                for g4 in range(gpd):
                    for cs in range(CS):
                        nc.vector.tensor_scalar_mul(
                            src[:, g4, cs, :], qb[:, cs, :],
                            st[:, cs, g4:g4 + 1])

            def emit_one(r, qb):
                # out[gt*gpd+g4, 3*gp+cs, f]: tile gt is one DMA of
                # gpd*0.75 MiB; partition gp's run for (g4, cs) is 2 KiB.
                out_r = out_views[r].rearrange(
                    "(gt g4) (gp cs) f -> gt gp g4 cs f", g4=gpd, cs=CS)
                for t in range(NT):
                    dma_eng = nc.scalar if (dual_ring and t % 2) else nc.sync
                    if dma_pure:
                        dma_eng.dma_start(out=out_r[t], in_=src[:])
                        continue
                    ot = wpool.tile([P, gpd, CS, HW], odt, name="ot", tag="ot")
                    for g4 in range(gpd):
                        n = t * gpd + g4
                        for cs in range(CS):
                            nc.vector.tensor_scalar_mul(
                                ot[:, g4, cs, :], qb[:, cs, :],
                                st[:, cs, n:n + 1])
                    if fine_edges and t in (0, NT - 1):
                        # Fill/drain in 0.25 MiB steps: the first store
                        # issues after a single multiply; the final drain
                        # is one small store, not a 3 MiB one.
                        for g4 in range(gpd):
                            for cs in range(CS):
                                dma_eng.dma_start(
                                    out=out_r[t][:, g4, cs, :],
                                    in_=ot[:, g4, cs, :])
                    else:
                        dma_eng.dma_start(out=out_r[t], in_=ot[:])

            def emit_repeats():
                for r in range(repeat):
                    emit_one(r, load_q() if full_body else qb)

            if timing and outer:
                with tc.For_i(0, outer, 1):
                    emit_repeats()
            else:
                emit_repeats()

            if timing:
                nc.sync.dma_start(out=tiny[:], in_=scratch[:, 0, 0, 0:4])

    nc.compile()
    return nc


def make_in_maps(support_fibers, query_features, diag_weight,
                 q_bf16: bool = True):
    qf = np.asarray(query_features, dtype=np.float32).reshape(B, C, HW)
    if q_bf16:
        import ml_dtypes
        qf = qf.astype(ml_dtypes.bfloat16)
    qf = np.ascontiguousarray(qf)
    sfm = np.ascontiguousarray(np.asarray(support_fibers, dtype=np.float32))
    dwm = np.ascontiguousarray(
        np.asarray(diag_weight, dtype=np.float32).reshape(C, 1))
    return [{"q": qf[b], "sf": sfm, "dw": dwm} for b in range(B)]


_state = {}


def _ensure_exec():
    """Build the Bass program once and wrap it in a reusable jitted SPMD
    callable (same ``bass_exec`` primitive / NEFF as
    ``bass_utils.run_bass_kernel_spmd``, which re-traces and re-uploads
    the zero output buffers on every call). The donated zero output
    buffers are created on-device by a separate tiny jit."""
    if "exec" in _state:
        return
    import jax
    import jax.numpy as jnp
    from jax.experimental.shard_map import shard_map
    from jax.sharding import Mesh, NamedSharding, PartitionSpec

    import concourse.mybir as mybir
    from concourse import bass2jax

    nc = build_nat()
    bass2jax.install_neuronx_cc_hook()

    partition_name = nc.partition_id_tensor.name if nc.partition_id_tensor else None
    in_names, out_names, out_avals = [], [], []
    for alloc in nc.m.functions[0].allocations:
        if not isinstance(alloc, mybir.MemoryLocationSet):
            continue
        name = alloc.memorylocations[0].name
        if alloc.kind == "ExternalInput":
            if name != partition_name:
                in_names.append(name)
        elif alloc.kind == "ExternalOutput":
            out_names.append(name)
            out_avals.append(jax.core.ShapedArray(
                tuple(alloc.tensor_shape), mybir.dt.np(alloc.dtype)))
    n_params = len(in_names)
    all_in_names = list(in_names) + list(out_names)
    if partition_name is not None:
        all_in_names.append(partition_name)

    def _body(*args):
        operands = list(args)
        if partition_name is not None:
            operands.append(bass2jax.partition_id_tensor())
        return tuple(bass2jax._bass_exec_p.bind(
            *operands,
            out_avals=tuple(out_avals),
            in_names=tuple(all_in_names),
            out_names=tuple(out_names),
            lowering_input_output_aliases=(),
            sim_require_finite=True,
            sim_require_nnan=True,
            nc=nc,
        ))

    devices = jax.devices()[:N_CORES]
    mesh = Mesh(np.asarray(devices), ("core",))
    n_outs = len(out_avals)
    sharded = jax.jit(
        shard_map(_body, mesh=mesh,
                  in_specs=(PartitionSpec("core"),) * (n_params + n_outs),
                  out_specs=(PartitionSpec("core"),) * n_outs,
                  check_rep=False),
        donate_argnums=tuple(range(n_params, n_params + n_outs)),
        keep_unused=True,
    )
    sh = NamedSharding(mesh, PartitionSpec("core"))
    zero_shapes = [(N_CORES * a.shape[0], *a.shape[1:]) for a in out_avals]
    zeros_fn = jax.jit(
        lambda: tuple(jnp.zeros(s, a.dtype)
                      for s, a in zip(zero_shapes, out_avals)),
        out_shardings=(sh,) * n_outs)

    _state.update(nc=nc, exec=sharded, zeros=zeros_fn, sharding=sh,
                  in_names=in_names)


def _fast_call(in_maps):
    from concurrent.futures import ThreadPoolExecutor

    import jax

    in_names = _state["in_names"]
    sh = _state["sharding"]
    concat_in = [
        jax.device_put(
            np.concatenate([np.asarray(m[name]) for m in in_maps], axis=0), sh)
        for name in in_names
    ]
    zeros = _state["zeros"]()
    out = _state["exec"](*concat_in, *zeros)[0]  # [N_CORES*NP, C, HW] bf16
    shards = sorted(out.addressable_shards,
                    key=lambda s: s.index[0].start or 0)
    if len(shards) == N_CORES:
        with ThreadPoolExecutor(N_CORES) as ex:
            parts = list(ex.map(
                lambda s: np.asarray(s.data).astype(np.float32), shards))
        return np.concatenate(parts, axis=0)
    return np.asarray(out).astype(np.float32)


def kernel(support_fibers, query_features, diag_weight):
    try:
        in_maps = make_in_maps(support_fibers, query_features, diag_weight)
        _ensure_exec()
        flat = _fast_call(in_maps)
    except Exception:
        from concourse.bass_utils import run_bass_kernel_spmd

        in_maps = make_in_maps(support_fibers, query_features, diag_weight,
                               q_bf16=False)
        nc = _state.get("nc_legacy")
        if nc is None:
            nc = build()
            _state["nc_legacy"] = nc
        res = run_bass_kernel_spmd(nc, in_maps, core_ids=list(range(N_CORES)))
        flat = np.concatenate(
            [np.asarray(res.results[b]["out"]).astype(np.float32)
             for b in range(B)], axis=0)
    return flat.reshape(B, NP, C, H, W)



# revision 18
# speedup vs baseline: 1.0001x; 1.0001x over previous
"""Trainium2 Bass kernel for nn_AttentionBasedModulator.

Computes out[b, n, c, h, w] = query_features[b, c, h, w]
                              * support_fibers[c, n] * diag_weight[c]

Sharding: data-parallel over batch B=8, one batch element per NeuronCore.

The kernel is DMA-write bound: the output is 32x larger than the input
(pure broadcast expansion), so per core the fp32 output would be
48 MiB vs ~1.5 MiB of input. The device computes and stores the output
in bf16 (24 MiB per core; worst-case rel err ~0.8%, measured 0.0077 vs
the 2e-2 gate) and the host upcasts to fp32 while gathering. At bf16
the steady state sits exactly on the HBM-per-NeuronCore write limit
(~358 GB/s = 716 GB/s per HBM stack shared by 2 cores): 24 MiB /
70 us = 359 GB/s. Three probes pin this as the physical roofline: a
pure-DMA store loop (dma_pure) times identically, fp32 output times
exactly 2x, and every layout/queue variant (gpd 2/4/8, dual-ring,
dup-layouts) lands in the same 69-72 us noise band. fp8 would halve
the bytes but cannot pass the gate (e4m3 rounding alone is 6.25%).

Primary builder build_nat (natural layout):
  - partition p holds channels 3p..3p+2 (CS = C/128 = 3); no q
    duplication, no SBUF replication step.
  - q arrives from the host already cast to bf16 (make_in_maps;
    identical rounding to the on-device cast it replaces), halving the
    q read and removing the cast from the prologue's critical path.
  - s tile [128, 3, 32] fp32: st[p, k, n] = sf[3p+k, n] * dw[3p+k],
    loaded via strided view, scaled on-chip. Kept fp32 (the
    tensor_scalar scalar operand may be fp32 in any DVE perf mode).
  - one output DMA covers gpd=4 prototypes through a 4-dim DRAM view
    out[gt, gp, g4, cs, f]: 3 MiB per DMA, 2 KiB contiguous per
    (partition, g4, cs) run — comfortably above the ~0.7 KiB where
    descriptor overhead would eat the 435 (fabric) / 358 (HBM) slack.
  - per tile: 12 DVE tensor_scalar multiplies (bf16 4x perf mode) into
    a bf16 tile, then the store; 96 multiplies ~18 us, hidden under
    70 us of DMA.
  - fine_edges: first/last tile stored in per-(g4, cs) 0.25 MiB steps,
    so the first store issues ~2 us into the program (after one 0.25 MiB
    q load + one multiply) and the final drain is short. The full-body
    probe (q load + all stores per iteration, inner=1 so the For_i
    back-edge exposes fill/drain) times the real single-shot NEFF body
    at ~79 us vs ~103 us for the legacy dup-layout build, whose
    whole-tile SBUF replication barrier serialized ~18 us of prologue.

The legacy dup-layout build() is kept as the fallback path.
"""

import numpy as np

C, NP = 384, 32          # channels, prototypes
B, H, W = 8, 32, 32
HW = H * W
P = 128                  # SBUF partitions
N_CORES = 8
DUP = 4                  # q duplication factor / prototypes per output DMA
BUFS = 4                 # output tile slots
ACT_SPLIT = 0            # of each group's CSD multiplies, how many go
                         # to the ACT (scalar) engine instead of the DVE


def build(repeat: int = 1, timing: bool = False, outer: int = 0,
          dup: int = DUP, onchip_dup: bool = True,
          bufs: int = BUFS, act_split: int = ACT_SPLIT, dma_lite: bool = False,
          scratch_regions: int = 5,
          dma_pure: bool = False, dual_ring: bool = False,
          split_half: bool = False, full_body: bool = False,
          fine_edges: bool = True, out_f32: bool = False):
    """Build and compile the Bass program for one core.

    timing=True: each repeat writes a distinct Internal DRAM region (so
    stores cannot be dead-store-eliminated); a final DRAM->DRAM readback
    of a few bytes per region forms the only ExternalOutput, so dispatch
    timing is not dominated by fetching the full output to the host.
    outer=N (with timing): wraps the python-unrolled `repeat` body in a
    hardware For_i loop of N iterations, giving N*repeat total repeats at
    fixed compile cost - long device programs make wall-clock timing
    robust to dispatch-overhead noise. The loop back-edge costs one
    drain+barrier per `repeat` repeats (amortized, slightly conservative).
    dup: q duplication factor (1, 2, or 4) - prototypes per output DMA;
    per-partition contiguous DRAM runs are dup*3 KiB (bf16).
    onchip_dup=True: load q from HBM once (1.5 MiB) into partition block
    0 and replicate across the dup blocks by log2-doubling SBUF->SBUF
    DMA copies, instead of dup HBM loads - cuts the real (repeat=1)
    kernel's HBM read traffic by (dup-1)*1.5 MiB; no effect on the
    steady-state repeat loop.
    dma_lite=True: only one multiply per output tile (rest of the tile is
    stale slot data) - isolates DMA-write throughput from DVE work.
    out_f32=True: fp32 output path (for A/B against the bf16 one).
    """
    import concourse.bacc as bacc
    import concourse.mybir as mybir
    from concourse.tile import TileContext

    nc = bacc.Bacc(None, target_bir_lowering=False)
    f32 = mybir.dt.float32
    bf16 = mybir.dt.bfloat16
    odt = f32 if out_f32 else bf16
    act_copy = mybir.ActivationFunctionType.Copy

    GP = P // dup            # partition groups (64 for dup=2)
    CSD = C // GP            # channels per partition (6 for dup=2)
    NPG = NP // dup          # prototype groups = output DMAs per repeat

    q = nc.dram_tensor("q", [C, HW], f32, kind="ExternalInput")
    sf = nc.dram_tensor("sf", [C, NP], f32, kind="ExternalInput")
    dw = nc.dram_tensor("dw", [C, 1], f32, kind="ExternalInput")
    if timing:
        nreg = min(repeat, scratch_regions)
        scratch = nc.dram_tensor("scratch", [nreg, NP, C, HW], odt,
                                 kind="Internal")
        tiny = nc.dram_tensor("out", [nreg, 4], odt, kind="ExternalOutput")
        out_views = [scratch[r % nreg] for r in range(repeat)]
    else:
        out = nc.dram_tensor("out", [NP, C, HW], odt, kind="ExternalOutput")
        tiny = None
        out_views = [out] * repeat

    # Grouped views: partition p = d*GP + gp <-> channels CSD*gp..CSD*gp+CSD-1
    # (the d halves hold IDENTICAL q channels but serve prototypes n = g*dup+d).
    q_r = q.rearrange("(gp cs) f -> gp cs f", cs=CSD)        # [GP, CSD, 1024]
    sf_r = sf.rearrange("(gp cs) (g d) -> d gp cs g",
                        cs=CSD, d=dup)                       # [dup,GP,CSD,NPG]
    dw_r = dw.rearrange("(gp cs) o -> gp cs o", cs=CSD)      # [GP, CSD, 1]

    with TileContext(nc) as tc:
        with tc.tile_pool(name="consts", bufs=1) as cpool, \
             tc.tile_pool(name="qpool", bufs=1) as qpool, \
             tc.tile_pool(name="work", bufs=bufs) as wpool:
            # Tiny sf/dw loads first: the s precompute overlaps the q load.
            st = cpool.tile([P, CSD, NPG], f32, name="st")
            for d in range(dup):
                nc.sync.dma_start(out=st[d * GP:(d + 1) * GP], in_=sf_r[d])
            dt_ = cpool.tile([P, CSD], f32, name="dt")
            for d in range(dup):
                nc.sync.dma_start(out=dt_[d * GP:(d + 1) * GP], in_=dw_r)

            def load_q():
                qt = qpool.tile([P, CSD, HW], f32, name="qt", tag="qt")
                qb = qt if out_f32 else qpool.tile([P, CSD, HW], bf16,
                                                   name="qb", tag="qb")
                if onchip_dup:
                    # One 1.5 MiB HBM load into partition block 0 (per-cs
                    # so casts overlap the loads), then replicate to the
                    # other dup-1 blocks by doubling SBUF->SBUF copies.
                    for cs in range(CSD):
                        nc.sync.dma_start(out=qt[0:GP, cs, :],
                                          in_=q_r[:, cs, :])
                        if not out_f32:
                            nc.vector.tensor_scalar_mul(qb[0:GP, cs, :],
                                                        qt[0:GP, cs, :], 1.0)
                    blk = GP
                    while blk < P:
                        nc.sync.dma_start(out=qb[blk:2 * blk], in_=qb[0:blk])
                        blk *= 2
                else:
                    # Per-(d, cs) loads let the first multiplies start
                    # sooner.
                    for cs in range(CSD):
                        for d in range(dup):
                            nc.sync.dma_start(
                                out=qt[d * GP:(d + 1) * GP, cs, :],
                                in_=q_r[:, cs, :])
                        if not out_f32:
                            nc.vector.tensor_scalar_mul(qb[:, cs, :],
                                                        qt[:, cs, :], 1.0)
                return qb

            for cs in range(CSD):
                nc.vector.tensor_scalar_mul(st[:, cs, :], st[:, cs, :],
                                            dt_[:, cs:cs + 1])

            qb = None if full_body else load_q()

            src = None
            if dma_pure:
                # One static source tile, filled once: the repeat loop is
                # pure independent DMA stores (measures the DMA ceiling).
                src = cpool.tile([P, CSD, HW], odt, name="src")
                for cs in range(CSD):
                    nc.vector.tensor_scalar_mul(
                        src[:, cs, :], qb[:, cs, :], st[:, cs, 0:1])

            def emit_repeats():
                for r in range(repeat):
                    emit_one(r, load_q() if full_body else qb)

            def emit_one(r, qb):
                # One DMA covers prototypes n = g*dup..g*dup+dup-1: partition
                # p = d*GP+gp writes the contiguous CSD-channel run of
                # prototype g*dup+d -> a single contiguous dup*768 KiB span.
                out_r = out_views[r].rearrange(
                    "(g d) (gp cs) f -> g (d gp) cs f", d=dup, cs=CSD)
                for g in range(NPG):
                    dma_eng = nc.scalar if (dual_ring and g % 2) else nc.sync
                    if dma_pure:
                        dma_eng.dma_start(out=out_r[g], in_=src[:])
                        continue
                    ot = wpool.tile([P, CSD, HW], odt, name="ot", tag="ot")
                    for cs in range(CSD):
                        if dma_lite and cs > 0:
                            continue
                        if cs < act_split:
                            nc.scalar.activation(
                                ot[:, cs, :], qb[:, cs, :], act_copy,
                                scale=st[:, cs, g:g + 1])
                        else:
                            nc.vector.tensor_scalar_mul(
                                ot[:, cs, :], qb[:, cs, :],
                                st[:, cs, g:g + 1])
                    if fine_edges and g in (0, NPG - 1) and not dma_lite:
                        # Fill/drain the pipeline in per-cs steps at the
                        # kernel edges: the first DMA starts after one
                        # multiply instead of CSD, and the final drain is
                        # 1/CSD as long.
                        for cs in range(CSD):
                            dma_eng.dma_start(out=out_r[g][:, cs, :],
                                              in_=ot[:, cs, :])
                    elif split_half:
                        # Same tile as two concurrent half-DMAs, one per
                        # HWDGE ring (partition halves map to disjoint
                        # SDMA engine sets).
                        nc.sync.dma_start(out=out_r[g][0:P // 2],
                                          in_=ot[0:P // 2])
                        nc.scalar.dma_start(out=out_r[g][P // 2:P],
                                            in_=ot[P // 2:P])
                    else:
                        dma_eng.dma_start(out=out_r[g], in_=ot[:])

            if timing and outer:
                with tc.For_i(0, outer, 1):
                    emit_repeats()
            else:
                emit_repeats()

            if timing:
                nc.sync.dma_start(out=tiny[:], in_=scratch[:, 0, 0, 0:4])

    nc.compile()
    return nc


GPD = 4                  # prototypes per output DMA tile (natural layout)
NAT_BUFS = 4


def build_nat(repeat: int = 1, timing: bool = False, outer: int = 0,
              gpd: int = GPD, bufs: int = NAT_BUFS,
              fine_edges: bool = True, full_body: bool = False,
              scratch_regions: int = 5, dma_pure: bool = False,
              dual_ring: bool = False, out_f32: bool = False,
              q_bf16: bool = True, fine_f: int = 1):
    """Natural-layout builder: partition p holds channels 3p..3p+2 (no q
    duplication), one output DMA covers `gpd` prototypes via a 4-dim DRAM
    access pattern out[gt, gp, g4, cs, f] (per-partition contiguous runs
    of 2 KiB at bf16 — well above the ~0.7 KiB where descriptor overhead
    would start to eat into the 435/358 fabric/HBM slack).

    The point vs the dup-layout build(): the prologue is a per-cs
    load -> cast -> (first-tile multiply + store) dataflow chain with no
    whole-tile replication barrier, so the first output DMA issues ~3 us
    into the program instead of ~18 us. Steady state is identical (HBM
    write roofline).

    full_body=True (timing only): the q load + cast runs inside every
    repeat, so with an INNER=1 hardware loop the per-iteration marginal
    time ~= the real single-shot NEFF body time (fill/drain included,
    exposed by the For_i back-edge drain).
    """
    import concourse.bacc as bacc
    import concourse.mybir as mybir
    from concourse.tile import TileContext

    nc = bacc.Bacc(None, target_bir_lowering=False)
    f32 = mybir.dt.float32
    bf16 = mybir.dt.bfloat16
    odt = f32 if out_f32 else bf16

    CS = C // P              # 3 channels per partition
    NT = NP // gpd           # output DMA tiles per repeat

    if out_f32:
        q_bf16 = False
    # q arrives pre-cast to bf16 by make_in_maps (identical rounding to
    # the on-device cast it replaces): halves the q read and drops the
    # cast + f32 staging tile from the prologue's critical path.
    q = nc.dram_tensor("q", [C, HW], bf16 if q_bf16 else f32,
                       kind="ExternalInput")
    sf = nc.dram_tensor("sf", [C, NP], f32, kind="ExternalInput")
    dw = nc.dram_tensor("dw", [C, 1], f32, kind="ExternalInput")
    if timing:
        nreg = min(repeat, scratch_regions)
        scratch = nc.dram_tensor("scratch", [nreg, NP, C, HW], odt,
                                 kind="Internal")
        tiny = nc.dram_tensor("out", [nreg, 4], odt, kind="ExternalOutput")
        out_views = [scratch[r % nreg] for r in range(repeat)]
    else:
        out = nc.dram_tensor("out", [NP, C, HW], odt, kind="ExternalOutput")
        tiny = None
        out_views = [out] * repeat

    q_r = q.rearrange("(gp cs) f -> gp cs f", cs=CS)        # [128, 3, 1024]
    sf_r = sf.rearrange("(gp cs) n -> gp cs n", cs=CS)      # [128, 3, 32]
    dw_r = dw.rearrange("(gp cs) o -> gp cs o", cs=CS)      # [128, 3, 1]

    with TileContext(nc) as tc:
        with tc.tile_pool(name="consts", bufs=1) as cpool, \
             tc.tile_pool(name="qpool", bufs=(2 if full_body else 1)) as qpool, \
             tc.tile_pool(name="work", bufs=bufs) as wpool:
            st = cpool.tile([P, CS, NP], f32, name="st")
            nc.sync.dma_start(out=st[:], in_=sf_r)
            dt_ = cpool.tile([P, CS], f32, name="dt")
            nc.sync.dma_start(out=dt_[:], in_=dw_r)
            for cs in range(CS):
                nc.vector.tensor_scalar_mul(st[:, cs, :], st[:, cs, :],
                                            dt_[:, cs:cs + 1])

            def load_q():
                # Per-cs loads (+casts if q arrives f32): each is an
                # independent dataflow chain, so tile 0's multiply/store
                # for cs starts as soon as THAT cs has landed. The very
                # first cs arrives in fine_f f-chunks (64 KiB at fine_f=4)
                # so the first multiply fires ~0.5 us after program start.
                if q_bf16:
                    qb = qpool.tile([P, CS, HW], bf16, name="qb", tag="qb")
                    for cs in range(CS):
                        if cs == 0 and fine_f > 1 and fine_edges:
                            fstep = HW // fine_f
                            for fi in range(fine_f):
                                fs = fi * fstep
                                nc.sync.dma_start(
                                    out=qb[:, 0, fs:fs + fstep],
                                    in_=q_r[:, 0, fs:fs + fstep])
                        else:
                            nc.sync.dma_start(out=qb[:, cs, :],
                                              in_=q_r[:, cs, :])
                    return qb
                qt = qpool.tile([P, CS, HW], f32, name="qt", tag="qt")
                qb = qt if out_f32 else qpool.tile([P, CS, HW], bf16,
                                                   name="qb", tag="qb")
                for cs in range(CS):
                    nc.sync.dma_start(out=qt[:, cs, :], in_=q_r[:, cs, :])
                    if not out_f32:
                        nc.vector.tensor_scalar_mul(qb[:, cs, :],
                                                    qt[:, cs, :], 1.0)
                return qb

            qb = None if full_body else load_q()

            src = None
            if dma_pure:
                src = cpool.tile([P, gpd, CS, HW], odt, name="src")
                for g4 in range(gpd):
                    for cs in range(CS):
                        nc.vector.tensor_scalar_mul(
                            src[:, g4, cs, :], qb[:, cs, :],
                            st[:, cs, g4:g4 + 1])

            def emit_one(r, qb):
                # out[gt*gpd+g4, 3*gp+cs, f]: tile gt is one DMA of
                # gpd*0.75 MiB; partition gp's run for (g4, cs) is 2 KiB.
                out_r = out_views[r].rearrange(
                    "(gt g4) (gp cs) f -> gt gp g4 cs f", g4=gpd, cs=CS)
                for t in range(NT):
                    dma_eng = nc.scalar if (dual_ring and t % 2) else nc.sync
                    if dma_pure:
                        dma_eng.dma_start(out=out_r[t], in_=src[:])
                        continue
                    ot = wpool.tile([P, gpd, CS, HW], odt, name="ot", tag="ot")
                    for g4 in range(gpd):
                        n = t * gpd + g4
                        for cs in range(CS):
                            if (t == 0 and g4 == 0 and cs == 0
                                    and fine_edges and fine_f > 1):
                                # Chunked head multiply: each f-chunk only
                                # needs its own slice of the q load.
                                fstep = HW // fine_f
                                for fi in range(fine_f):
                                    fs = fi * fstep
                                    nc.vector.tensor_scalar_mul(
                                        ot[:, 0, 0, fs:fs + fstep],
                                        qb[:, 0, fs:fs + fstep],
                                        st[:, 0, n:n + 1])
                            else:
                                nc.vector.tensor_scalar_mul(
                                    ot[:, g4, cs, :], qb[:, cs, :],
                                    st[:, cs, n:n + 1])
                    if fine_edges and t in (0, NT - 1):
                        # Fill/drain in 0.25 MiB steps: the first store
                        # issues after a single multiply; the final drain
                        # is one small store, not a 3 MiB one. The very
                        # first and very last (g4, cs) go in fine_f
                        # f-chunks so the pipeline head/tail transients
                        # are ~64 KiB, not 0.25 MiB.
                        for g4 in range(gpd):
                            for cs in range(CS):
                                head = t == 0 and g4 == 0 and cs == 0
                                tail = (t == NT - 1 and g4 == gpd - 1
                                        and cs == CS - 1)
                                if (head or tail) and fine_f > 1:
                                    fstep = HW // fine_f
                                    for fi in range(fine_f):
                                        fs = fi * fstep
                                        dma_eng.dma_start(
                                            out=out_r[t][:, g4, cs,
                                                         fs:fs + fstep],
                                            in_=ot[:, g4, cs,
                                                   fs:fs + fstep])
                                else:
                                    dma_eng.dma_start(
                                        out=out_r[t][:, g4, cs, :],
                                        in_=ot[:, g4, cs, :])
                    else:
                        dma_eng.dma_start(out=out_r[t], in_=ot[:])

            def emit_repeats():
                for r in range(repeat):
                    emit_one(r, load_q() if full_body else qb)

            if timing and outer:
                with tc.For_i(0, outer, 1):
                    emit_repeats()
            else:
                emit_repeats()

            if timing:
                nc.sync.dma_start(out=tiny[:], in_=scratch[:, 0, 0, 0:4])

    nc.compile()
    return nc


def make_in_maps(support_fibers, query_features, diag_weight,
                 q_bf16: bool = True):
    qf = np.asarray(query_features, dtype=np.float32).reshape(B, C, HW)
    if q_bf16:
        import ml_dtypes
        qf = qf.astype(ml_dtypes.bfloat16)
    qf = np.ascontiguousarray(qf)
    sfm = np.ascontiguousarray(np.asarray(support_fibers, dtype=np.float32))
    dwm = np.ascontiguousarray(
        np.asarray(diag_weight, dtype=np.float32).reshape(C, 1))
    return [{"q": qf[b], "sf": sfm, "dw": dwm} for b in range(B)]


_state = {}


def _ensure_exec():
    """Build the Bass program once and wrap it in a reusable jitted SPMD
    callable (same ``bass_exec`` primitive / NEFF as
    ``bass_utils.run_bass_kernel_spmd``, which re-traces and re-uploads
    the zero output buffers on every call). The donated zero output
    buffers are created on-device by a separate tiny jit."""
    if "exec" in _state:
        return
    import jax
    import jax.numpy as jnp
    from jax.experimental.shard_map import shard_map
    from jax.sharding import Mesh, NamedSharding, PartitionSpec

    import concourse.mybir as mybir
    from concourse import bass2jax

    nc = build_nat()
    bass2jax.install_neuronx_cc_hook()

    partition_name = nc.partition_id_tensor.name if nc.partition_id_tensor else None
    in_names, out_names, out_avals = [], [], []
    for alloc in nc.m.functions[0].allocations:
        if not isinstance(alloc, mybir.MemoryLocationSet):
            continue
        name = alloc.memorylocations[0].name
        if alloc.kind == "ExternalInput":
            if name != partition_name:
                in_names.append(name)
        elif alloc.kind == "ExternalOutput":
            out_names.append(name)
            out_avals.append(jax.core.ShapedArray(
                tuple(alloc.tensor_shape), mybir.dt.np(alloc.dtype)))
    n_params = len(in_names)
    all_in_names = list(in_names) + list(out_names)
    if partition_name is not None:
        all_in_names.append(partition_name)

    def _body(*args):
        operands = list(args)
        if partition_name is not None:
            operands.append(bass2jax.partition_id_tensor())
        return tuple(bass2jax._bass_exec_p.bind(
            *operands,
            out_avals=tuple(out_avals),
            in_names=tuple(all_in_names),
            out_names=tuple(out_names),
            lowering_input_output_aliases=(),
            sim_require_finite=True,
            sim_require_nnan=True,
            nc=nc,
        ))

    devices = jax.devices()[:N_CORES]
    mesh = Mesh(np.asarray(devices), ("core",))
    n_outs = len(out_avals)
    sharded = jax.jit(
        shard_map(_body, mesh=mesh,
                  in_specs=(PartitionSpec("core"),) * (n_params + n_outs),
                  out_specs=(PartitionSpec("core"),) * n_outs,
                  check_rep=False),
        donate_argnums=tuple(range(n_params, n_params + n_outs)),
        keep_unused=True,
    )
    sh = NamedSharding(mesh, PartitionSpec("core"))
    zero_shapes = [(N_CORES * a.shape[0], *a.shape[1:]) for a in out_avals]
    zeros_fn = jax.jit(
        lambda: tuple(jnp.zeros(s, a.dtype)
                      for s, a in zip(zero_shapes, out_avals)),
        out_shardings=(sh,) * n_outs)

    _state.update(nc=nc, exec=sharded, zeros=zeros_fn, sharding=sh,
                  in_names=in_names)


def _fast_call(in_maps):
    from concurrent.futures import ThreadPoolExecutor

    import jax

    in_names = _state["in_names"]
    sh = _state["sharding"]
    concat_in = [
        jax.device_put(
            np.concatenate([np.asarray(m[name]) for m in in_maps], axis=0), sh)
        for name in in_names
    ]
    zeros = _state["zeros"]()
    out = _state["exec"](*concat_in, *zeros)[0]  # [N_CORES*NP, C, HW] bf16
    shards = sorted(out.addressable_shards,
                    key=lambda s: s.index[0].start or 0)
    if len(shards) == N_CORES:
        with ThreadPoolExecutor(N_CORES) as ex:
            parts = list(ex.map(
                lambda s: np.asarray(s.data).astype(np.float32), shards))
        return np.concatenate(parts, axis=0)
    return np.asarray(out).astype(np.float32)


def kernel(support_fibers, query_features, diag_weight):
    try:
        in_maps = make_in_maps(support_fibers, query_features, diag_weight)
        _ensure_exec()
        flat = _fast_call(in_maps)
    except Exception:
        from concourse.bass_utils import run_bass_kernel_spmd

        in_maps = make_in_maps(support_fibers, query_features, diag_weight,
                               q_bf16=False)
        nc = _state.get("nc_legacy")
        if nc is None:
            nc = build()
            _state["nc_legacy"] = nc
        res = run_bass_kernel_spmd(nc, in_maps, core_ids=list(range(N_CORES)))
        flat = np.concatenate(
            [np.asarray(res.results[b]["out"]).astype(np.float32)
             for b in range(B)], axis=0)
    return flat.reshape(B, NP, C, H, W)



# revision 21
# speedup vs baseline: 1.0045x; 1.0044x over previous
"""Trainium2 Bass kernel for nn_AttentionBasedModulator.

Computes out[b, n, c, h, w] = query_features[b, c, h, w]
                              * support_fibers[c, n] * diag_weight[c]

Sharding: data-parallel over batch B=8, one batch element per NeuronCore.

The kernel is DMA-write bound: the output is 32x larger than the input
(pure broadcast expansion), so per core the fp32 output would be
48 MiB vs ~1.5 MiB of input. The device computes and stores the output
in bf16 (24 MiB per core; worst-case rel err ~0.8%, measured 0.0077 vs
the 2e-2 gate) and the host upcasts to fp32 while gathering. At bf16
the steady state sits exactly on the HBM-per-NeuronCore write limit
(~358 GB/s = 716 GB/s per HBM stack shared by 2 cores): 24 MiB /
70 us = 359 GB/s. Three probes pin this as the physical roofline: a
pure-DMA store loop (dma_pure) times identically, fp32 output times
exactly 2x, and every layout/queue variant (gpd 2/4/8, dual-ring,
dup-layouts) lands in the same 69-72 us noise band. fp8 would halve
the bytes but cannot pass the gate (e4m3 rounding alone is 6.25%).

Primary builder build_nat (natural layout):
  - partition p holds channels 3p..3p+2 (CS = C/128 = 3); no q
    duplication, no SBUF replication step.
  - q arrives from the host already cast to bf16 (make_in_maps;
    identical rounding to the on-device cast it replaces), halving the
    q read and removing the cast from the prologue's critical path.
  - s tile [128, 3, 32] fp32: st[p, k, n] = sf[3p+k, n] * dw[3p+k],
    loaded via strided view, scaled on-chip. Kept fp32 (the
    tensor_scalar scalar operand may be fp32 in any DVE perf mode).
  - one output DMA covers gpd=4 prototypes through a 4-dim DRAM view
    out[gt, gp, g4, cs, f]: 3 MiB per DMA, 2 KiB contiguous per
    (partition, g4, cs) run — comfortably above the ~0.7 KiB where
    descriptor overhead would eat the 435 (fabric) / 358 (HBM) slack.
  - per tile: 12 DVE tensor_scalar multiplies (bf16 4x perf mode) into
    a bf16 tile, then the store; 96 multiplies ~18 us, hidden under
    70 us of DMA.
  - fine_edges: first/last tile stored in per-(g4, cs) 0.25 MiB steps,
    so the first store issues ~2 us into the program (after one 0.25 MiB
    q load + one multiply) and the final drain is short. The full-body
    probe (q load + all stores per iteration, inner=1 so the For_i
    back-edge exposes fill/drain) times the real single-shot NEFF body
    at ~79 us vs ~103 us for the legacy dup-layout build, whose
    whole-tile SBUF replication barrier serialized ~18 us of prologue.

The legacy dup-layout build() is kept as the fallback path.
"""

import numpy as np

C, NP = 384, 32          # channels, prototypes
B, H, W = 8, 32, 32
HW = H * W
P = 128                  # SBUF partitions
N_CORES = 8
DUP = 4                  # q duplication factor / prototypes per output DMA
BUFS = 4                 # output tile slots
ACT_SPLIT = 0            # of each group's CSD multiplies, how many go
                         # to the ACT (scalar) engine instead of the DVE


def build(repeat: int = 1, timing: bool = False, outer: int = 0,
          dup: int = DUP, onchip_dup: bool = True,
          bufs: int = BUFS, act_split: int = ACT_SPLIT, dma_lite: bool = False,
          scratch_regions: int = 5,
          dma_pure: bool = False, dual_ring: bool = False,
          split_half: bool = False, full_body: bool = False,
          fine_edges: bool = True, out_f32: bool = False):
    """Build and compile the Bass program for one core.

    timing=True: each repeat writes a distinct Internal DRAM region (so
    stores cannot be dead-store-eliminated); a final DRAM->DRAM readback
    of a few bytes per region forms the only ExternalOutput, so dispatch
    timing is not dominated by fetching the full output to the host.
    outer=N (with timing): wraps the python-unrolled `repeat` body in a
    hardware For_i loop of N iterations, giving N*repeat total repeats at
    fixed compile cost - long device programs make wall-clock timing
    robust to dispatch-overhead noise. The loop back-edge costs one
    drain+barrier per `repeat` repeats (amortized, slightly conservative).
    dup: q duplication factor (1, 2, or 4) - prototypes per output DMA;
    per-partition contiguous DRAM runs are dup*3 KiB (bf16).
    onchip_dup=True: load q from HBM once (1.5 MiB) into partition block
    0 and replicate across the dup blocks by log2-doubling SBUF->SBUF
    DMA copies, instead of dup HBM loads - cuts the real (repeat=1)
    kernel's HBM read traffic by (dup-1)*1.5 MiB; no effect on the
    steady-state repeat loop.
    dma_lite=True: only one multiply per output tile (rest of the tile is
    stale slot data) - isolates DMA-write throughput from DVE work.
    out_f32=True: fp32 output path (for A/B against the bf16 one).
    """
    import concourse.bacc as bacc
    import concourse.mybir as mybir
    from concourse.tile import TileContext

    nc = bacc.Bacc(None, target_bir_lowering=False)
    f32 = mybir.dt.float32
    bf16 = mybir.dt.bfloat16
    odt = f32 if out_f32 else bf16
    act_copy = mybir.ActivationFunctionType.Copy

    GP = P // dup            # partition groups (64 for dup=2)
    CSD = C // GP            # channels per partition (6 for dup=2)
    NPG = NP // dup          # prototype groups = output DMAs per repeat

    q = nc.dram_tensor("q", [C, HW], f32, kind="ExternalInput")
    sf = nc.dram_tensor("sf", [C, NP], f32, kind="ExternalInput")
    dw = nc.dram_tensor("dw", [C, 1], f32, kind="ExternalInput")
    if timing:
        nreg = min(repeat, scratch_regions)
        scratch = nc.dram_tensor("scratch", [nreg, NP, C, HW], odt,
                                 kind="Internal")
        tiny = nc.dram_tensor("out", [nreg, 4], odt, kind="ExternalOutput")
        out_views = [scratch[r % nreg] for r in range(repeat)]
    else:
        out = nc.dram_tensor("out", [NP, C, HW], odt, kind="ExternalOutput")
        tiny = None
        out_views = [out] * repeat

    # Grouped views: partition p = d*GP + gp <-> channels CSD*gp..CSD*gp+CSD-1
    # (the d halves hold IDENTICAL q channels but serve prototypes n = g*dup+d).
    q_r = q.rearrange("(gp cs) f -> gp cs f", cs=CSD)        # [GP, CSD, 1024]
    sf_r = sf.rearrange("(gp cs) (g d) -> d gp cs g",
                        cs=CSD, d=dup)                       # [dup,GP,CSD,NPG]
    dw_r = dw.rearrange("(gp cs) o -> gp cs o", cs=CSD)      # [GP, CSD, 1]

    with TileContext(nc) as tc:
        with tc.tile_pool(name="consts", bufs=1) as cpool, \
             tc.tile_pool(name="qpool", bufs=1) as qpool, \
             tc.tile_pool(name="work", bufs=bufs) as wpool:
            # Tiny sf/dw loads first: the s precompute overlaps the q load.
            st = cpool.tile([P, CSD, NPG], f32, name="st")
            for d in range(dup):
                nc.sync.dma_start(out=st[d * GP:(d + 1) * GP], in_=sf_r[d])
            dt_ = cpool.tile([P, CSD], f32, name="dt")
            for d in range(dup):
                nc.sync.dma_start(out=dt_[d * GP:(d + 1) * GP], in_=dw_r)

            def load_q():
                qt = qpool.tile([P, CSD, HW], f32, name="qt", tag="qt")
                qb = qt if out_f32 else qpool.tile([P, CSD, HW], bf16,
                                                   name="qb", tag="qb")
                if onchip_dup:
                    # One 1.5 MiB HBM load into partition block 0 (per-cs
                    # so casts overlap the loads), then replicate to the
                    # other dup-1 blocks by doubling SBUF->SBUF copies.
                    for cs in range(CSD):
                        nc.sync.dma_start(out=qt[0:GP, cs, :],
                                          in_=q_r[:, cs, :])
                        if not out_f32:
                            nc.vector.tensor_scalar_mul(qb[0:GP, cs, :],
                                                        qt[0:GP, cs, :], 1.0)
                    blk = GP
                    while blk < P:
                        nc.sync.dma_start(out=qb[blk:2 * blk], in_=qb[0:blk])
                        blk *= 2
                else:
                    # Per-(d, cs) loads let the first multiplies start
                    # sooner.
                    for cs in range(CSD):
                        for d in range(dup):
                            nc.sync.dma_start(
                                out=qt[d * GP:(d + 1) * GP, cs, :],
                                in_=q_r[:, cs, :])
                        if not out_f32:
                            nc.vector.tensor_scalar_mul(qb[:, cs, :],
                                                        qt[:, cs, :], 1.0)
                return qb

            for cs in range(CSD):
                nc.vector.tensor_scalar_mul(st[:, cs, :], st[:, cs, :],
                                            dt_[:, cs:cs + 1])

            qb = None if full_body else load_q()

            src = None
            if dma_pure:
                # One static source tile, filled once: the repeat loop is
                # pure independent DMA stores (measures the DMA ceiling).
                src = cpool.tile([P, CSD, HW], odt, name="src")
                for cs in range(CSD):
                    nc.vector.tensor_scalar_mul(
                        src[:, cs, :], qb[:, cs, :], st[:, cs, 0:1])

            def emit_repeats():
                for r in range(repeat):
                    emit_one(r, load_q() if full_body else qb)

            def emit_one(r, qb):
                # One DMA covers prototypes n = g*dup..g*dup+dup-1: partition
                # p = d*GP+gp writes the contiguous CSD-channel run of
                # prototype g*dup+d -> a single contiguous dup*768 KiB span.
                out_r = out_views[r].rearrange(
                    "(g d) (gp cs) f -> g (d gp) cs f", d=dup, cs=CSD)
                for g in range(NPG):
                    dma_eng = nc.scalar if (dual_ring and g % 2) else nc.sync
                    if dma_pure:
                        dma_eng.dma_start(out=out_r[g], in_=src[:])
                        continue
                    ot = wpool.tile([P, CSD, HW], odt, name="ot", tag="ot")
                    for cs in range(CSD):
                        if dma_lite and cs > 0:
                            continue
                        if cs < act_split:
                            nc.scalar.activation(
                                ot[:, cs, :], qb[:, cs, :], act_copy,
                                scale=st[:, cs, g:g + 1])
                        else:
                            nc.vector.tensor_scalar_mul(
                                ot[:, cs, :], qb[:, cs, :],
                                st[:, cs, g:g + 1])
                    if fine_edges and g in (0, NPG - 1) and not dma_lite:
                        # Fill/drain the pipeline in per-cs steps at the
                        # kernel edges: the first DMA starts after one
                        # multiply instead of CSD, and the final drain is
                        # 1/CSD as long.
                        for cs in range(CSD):
                            dma_eng.dma_start(out=out_r[g][:, cs, :],
                                              in_=ot[:, cs, :])
                    elif split_half:
                        # Same tile as two concurrent half-DMAs, one per
                        # HWDGE ring (partition halves map to disjoint
                        # SDMA engine sets).
                        nc.sync.dma_start(out=out_r[g][0:P // 2],
                                          in_=ot[0:P // 2])
                        nc.scalar.dma_start(out=out_r[g][P // 2:P],
                                            in_=ot[P // 2:P])
                    else:
                        dma_eng.dma_start(out=out_r[g], in_=ot[:])

            if timing and outer:
                with tc.For_i(0, outer, 1):
                    emit_repeats()
            else:
                emit_repeats()

            if timing:
                nc.sync.dma_start(out=tiny[:], in_=scratch[:, 0, 0, 0:4])

    nc.compile()
    return nc


GPD = 4                  # prototypes per output DMA tile (natural layout)
NAT_BUFS = 4


def build_nat(repeat: int = 1, timing: bool = False, outer: int = 0,
              gpd: int = GPD, bufs: int = NAT_BUFS,
              fine_edges: bool = True, full_body: bool = False,
              scratch_regions: int = 5, dma_pure: bool = False,
              dual_ring: bool = False, out_f32: bool = False,
              q_bf16: bool = True, fine_f: int = 1,
              fine_tiles: tuple | None = None):
    """Natural-layout builder: partition p holds channels 3p..3p+2 (no q
    duplication), one output DMA covers `gpd` prototypes via a 4-dim DRAM
    access pattern out[gt, gp, g4, cs, f] (per-partition contiguous runs
    of 2 KiB at bf16 — well above the ~0.7 KiB where descriptor overhead
    would start to eat into the 435/358 fabric/HBM slack).

    The point vs the dup-layout build(): the prologue is a per-cs
    load -> cast -> (first-tile multiply + store) dataflow chain with no
    whole-tile replication barrier, so the first output DMA issues ~3 us
    into the program instead of ~18 us. Steady state is identical (HBM
    write roofline).

    full_body=True (timing only): the q load + cast runs inside every
    repeat, so with an INNER=1 hardware loop the per-iteration marginal
    time ~= the real single-shot NEFF body time (fill/drain included,
    exposed by the For_i back-edge drain).
    """
    import concourse.bacc as bacc
    import concourse.mybir as mybir
    from concourse.tile import TileContext

    nc = bacc.Bacc(None, target_bir_lowering=False)
    f32 = mybir.dt.float32
    bf16 = mybir.dt.bfloat16
    odt = f32 if out_f32 else bf16

    CS = C // P              # 3 channels per partition
    NT = NP // gpd           # output DMA tiles per repeat
    if fine_tiles is None:
        fine_tiles = (0, NT - 1)
    fine_tiles = set(fine_tiles)

    if out_f32:
        q_bf16 = False
    # q arrives pre-cast to bf16 by make_in_maps (identical rounding to
    # the on-device cast it replaces): halves the q read and drops the
    # cast + f32 staging tile from the prologue's critical path.
    q = nc.dram_tensor("q", [C, HW], bf16 if q_bf16 else f32,
                       kind="ExternalInput")
    sf = nc.dram_tensor("sf", [C, NP], f32, kind="ExternalInput")
    dw = nc.dram_tensor("dw", [C, 1], f32, kind="ExternalInput")
    if timing:
        nreg = min(repeat, scratch_regions)
        scratch = nc.dram_tensor("scratch", [nreg, NP, C, HW], odt,
                                 kind="Internal")
        tiny = nc.dram_tensor("out", [nreg, 4], odt, kind="ExternalOutput")
        out_views = [scratch[r % nreg] for r in range(repeat)]
    else:
        out = nc.dram_tensor("out", [NP, C, HW], odt, kind="ExternalOutput")
        tiny = None
        out_views = [out] * repeat

    q_r = q.rearrange("(gp cs) f -> gp cs f", cs=CS)        # [128, 3, 1024]
    sf_r = sf.rearrange("(gp cs) n -> gp cs n", cs=CS)      # [128, 3, 32]
    dw_r = dw.rearrange("(gp cs) o -> gp cs o", cs=CS)      # [128, 3, 1]

    with TileContext(nc) as tc:
        with tc.tile_pool(name="consts", bufs=1) as cpool, \
             tc.tile_pool(name="qpool", bufs=(2 if full_body else 1)) as qpool, \
             tc.tile_pool(name="work", bufs=bufs) as wpool:
            st = cpool.tile([P, CS, NP], f32, name="st")
            nc.sync.dma_start(out=st[:], in_=sf_r)
            dt_ = cpool.tile([P, CS], f32, name="dt")
            nc.sync.dma_start(out=dt_[:], in_=dw_r)
            for cs in range(CS):
                nc.vector.tensor_scalar_mul(st[:, cs, :], st[:, cs, :],
                                            dt_[:, cs:cs + 1])

            def load_q():
                # Per-cs loads (+casts if q arrives f32): each is an
                # independent dataflow chain, so tile 0's multiply/store
                # for cs starts as soon as THAT cs has landed. The very
                # first cs arrives in fine_f f-chunks (64 KiB at fine_f=4)
                # so the first multiply fires ~0.5 us after program start.
                if q_bf16:
                    qb = qpool.tile([P, CS, HW], bf16, name="qb", tag="qb")
                    for cs in range(CS):
                        if cs == 0 and fine_f > 1 and fine_edges:
                            fstep = HW // fine_f
                            for fi in range(fine_f):
                                fs = fi * fstep
                                nc.sync.dma_start(
                                    out=qb[:, 0, fs:fs + fstep],
                                    in_=q_r[:, 0, fs:fs + fstep])
                        else:
                            nc.sync.dma_start(out=qb[:, cs, :],
                                              in_=q_r[:, cs, :])
                    return qb
                qt = qpool.tile([P, CS, HW], f32, name="qt", tag="qt")
                qb = qt if out_f32 else qpool.tile([P, CS, HW], bf16,
                                                   name="qb", tag="qb")
                for cs in range(CS):
                    nc.sync.dma_start(out=qt[:, cs, :], in_=q_r[:, cs, :])
                    if not out_f32:
                        nc.vector.tensor_scalar_mul(qb[:, cs, :],
                                                    qt[:, cs, :], 1.0)
                return qb

            qb = None if full_body else load_q()

            src = None
            if dma_pure:
                src = cpool.tile([P, gpd, CS, HW], odt, name="src")
                for g4 in range(gpd):
                    for cs in range(CS):
                        nc.vector.tensor_scalar_mul(
                            src[:, g4, cs, :], qb[:, cs, :],
                            st[:, cs, g4:g4 + 1])

            def emit_one(r, qb):
                # out[gt*gpd+g4, 3*gp+cs, f]: tile gt is one DMA of
                # gpd*0.75 MiB; partition gp's run for (g4, cs) is 2 KiB.
                out_r = out_views[r].rearrange(
                    "(gt g4) (gp cs) f -> gt gp g4 cs f", g4=gpd, cs=CS)
                for t in range(NT):
                    dma_eng = nc.scalar if (dual_ring and t % 2) else nc.sync
                    if dma_pure:
                        dma_eng.dma_start(out=out_r[t], in_=src[:])
                        continue
                    ot = wpool.tile([P, gpd, CS, HW], odt, name="ot", tag="ot")
                    for g4 in range(gpd):
                        n = t * gpd + g4
                        for cs in range(CS):
                            if (t == 0 and g4 == 0 and cs == 0
                                    and fine_edges and fine_f > 1):
                                # Chunked head multiply: each f-chunk only
                                # needs its own slice of the q load.
                                fstep = HW // fine_f
                                for fi in range(fine_f):
                                    fs = fi * fstep
                                    nc.vector.tensor_scalar_mul(
                                        ot[:, 0, 0, fs:fs + fstep],
                                        qb[:, 0, fs:fs + fstep],
                                        st[:, 0, n:n + 1])
                            else:
                                nc.vector.tensor_scalar_mul(
                                    ot[:, g4, cs, :], qb[:, cs, :],
                                    st[:, cs, n:n + 1])
                    if fine_edges and t in fine_tiles:
                        # Fill/drain in 0.25 MiB steps: the first store
                        # issues after a single multiply; the final drain
                        # is one small store, not a 3 MiB one. The very
                        # first and very last (g4, cs) go in fine_f
                        # f-chunks so the pipeline head/tail transients
                        # are ~64 KiB, not 0.25 MiB.
                        for g4 in range(gpd):
                            for cs in range(CS):
                                head = t == 0 and g4 == 0 and cs == 0
                                tail = (t == NT - 1 and g4 == gpd - 1
                                        and cs == CS - 1)
                                if (head or tail) and fine_f > 1:
                                    fstep = HW // fine_f
                                    for fi in range(fine_f):
                                        fs = fi * fstep
                                        dma_eng.dma_start(
                                            out=out_r[t][:, g4, cs,
                                                         fs:fs + fstep],
                                            in_=ot[:, g4, cs,
                                                   fs:fs + fstep])
                                else:
                                    dma_eng.dma_start(
                                        out=out_r[t][:, g4, cs, :],
                                        in_=ot[:, g4, cs, :])
                    else:
                        dma_eng.dma_start(out=out_r[t], in_=ot[:])

            def emit_repeats():
                for r in range(repeat):
                    emit_one(r, load_q() if full_body else qb)

            if timing and outer:
                with tc.For_i(0, outer, 1):
                    emit_repeats()
            else:
                emit_repeats()

            if timing:
                nc.sync.dma_start(out=tiny[:], in_=scratch[:, 0, 0, 0:4])

    nc.compile()
    return nc


def make_in_maps(support_fibers, query_features, diag_weight,
                 q_bf16: bool = True):
    qf = np.asarray(query_features, dtype=np.float32).reshape(B, C, HW)
    if q_bf16:
        import ml_dtypes
        qf = qf.astype(ml_dtypes.bfloat16)
    qf = np.ascontiguousarray(qf)
    sfm = np.ascontiguousarray(np.asarray(support_fibers, dtype=np.float32))
    dwm = np.ascontiguousarray(
        np.asarray(diag_weight, dtype=np.float32).reshape(C, 1))
    return [{"q": qf[b], "sf": sfm, "dw": dwm} for b in range(B)]


_state = {}


def _ensure_exec():
    """Build the Bass program once and wrap it in a reusable jitted SPMD
    callable (same ``bass_exec`` primitive / NEFF as
    ``bass_utils.run_bass_kernel_spmd``, which re-traces and re-uploads
    the zero output buffers on every call). The donated zero output
    buffers are created on-device by a separate tiny jit."""
    if "exec" in _state:
        return
    import jax
    import jax.numpy as jnp
    from jax.experimental.shard_map import shard_map
    from jax.sharding import Mesh, NamedSharding, PartitionSpec

    import concourse.mybir as mybir
    from concourse import bass2jax

    nc = build_nat()
    bass2jax.install_neuronx_cc_hook()

    partition_name = nc.partition_id_tensor.name if nc.partition_id_tensor else None
    in_names, out_names, out_avals = [], [], []
    for alloc in nc.m.functions[0].allocations:
        if not isinstance(alloc, mybir.MemoryLocationSet):
            continue
        name = alloc.memorylocations[0].name
        if alloc.kind == "ExternalInput":
            if name != partition_name:
                in_names.append(name)
        elif alloc.kind == "ExternalOutput":
            out_names.append(name)
            out_avals.append(jax.core.ShapedArray(
                tuple(alloc.tensor_shape), mybir.dt.np(alloc.dtype)))
    n_params = len(in_names)
    all_in_names = list(in_names) + list(out_names)
    if partition_name is not None:
        all_in_names.append(partition_name)

    def _body(*args):
        operands = list(args)
        if partition_name is not None:
            operands.append(bass2jax.partition_id_tensor())
        return tuple(bass2jax._bass_exec_p.bind(
            *operands,
            out_avals=tuple(out_avals),
            in_names=tuple(all_in_names),
            out_names=tuple(out_names),
            lowering_input_output_aliases=(),
            sim_require_finite=True,
            sim_require_nnan=True,
            nc=nc,
        ))

    devices = jax.devices()[:N_CORES]
    mesh = Mesh(np.asarray(devices), ("core",))
    n_outs = len(out_avals)
    sharded = jax.jit(
        shard_map(_body, mesh=mesh,
                  in_specs=(PartitionSpec("core"),) * (n_params + n_outs),
                  out_specs=(PartitionSpec("core"),) * n_outs,
                  check_rep=False),
        donate_argnums=tuple(range(n_params, n_params + n_outs)),
        keep_unused=True,
    )
    sh = NamedSharding(mesh, PartitionSpec("core"))
    zero_shapes = [(N_CORES * a.shape[0], *a.shape[1:]) for a in out_avals]
    zeros_fn = jax.jit(
        lambda: tuple(jnp.zeros(s, a.dtype)
                      for s, a in zip(zero_shapes, out_avals)),
        out_shardings=(sh,) * n_outs)

    _state.update(nc=nc, exec=sharded, zeros=zeros_fn, sharding=sh,
                  in_names=in_names)


def _fast_call(in_maps):
    from concurrent.futures import ThreadPoolExecutor

    import jax

    in_names = _state["in_names"]
    sh = _state["sharding"]
    concat_in = [
        jax.device_put(
            np.concatenate([np.asarray(m[name]) for m in in_maps], axis=0), sh)
        for name in in_names
    ]
    zeros = _state["zeros"]()
    out = _state["exec"](*concat_in, *zeros)[0]  # [N_CORES*NP, C, HW] bf16
    shards = sorted(out.addressable_shards,
                    key=lambda s: s.index[0].start or 0)
    if len(shards) == N_CORES:
        with ThreadPoolExecutor(N_CORES) as ex:
            parts = list(ex.map(
                lambda s: np.asarray(s.data).astype(np.float32), shards))
        return np.concatenate(parts, axis=0)
    return np.asarray(out).astype(np.float32)


def kernel(support_fibers, query_features, diag_weight):
    try:
        in_maps = make_in_maps(support_fibers, query_features, diag_weight)
        _ensure_exec()
        flat = _fast_call(in_maps)
    except Exception:
        from concourse.bass_utils import run_bass_kernel_spmd

        in_maps = make_in_maps(support_fibers, query_features, diag_weight,
                               q_bf16=False)
        nc = _state.get("nc_legacy")
        if nc is None:
            nc = build()
            _state["nc_legacy"] = nc
        res = run_bass_kernel_spmd(nc, in_maps, core_ids=list(range(N_CORES)))
        flat = np.concatenate(
            [np.asarray(res.results[b]["out"]).astype(np.float32)
             for b in range(B)], axis=0)
    return flat.reshape(B, NP, C, H, W)

